# revision 96
# baseline (speedup 1.0000x reference)
"""Trainium2 Bass kernel for sliding-window causal self-attention (GQA + RoPE +
QK-RMSNorm + value-embedding gate), sequence-sharded over 8 NeuronCores.

Shapes (hardcoded): B=1, T=4096, C=1024, H=16, HKV=4, D=64, window=1024.

Sharding: core i owns output rows [512*i, 512*(i+1)).  Each core recomputes
K/V for its 1024-row halo (rows [512*i-1024, 512*(i+1)), zero-padded below
row 0) so no collectives are needed.  Padded rows yield k_hat = 0 =>
exp(score)=1 exactly; a host-computed additive denominator correction
removes those contributions.

v2 design notes (vs the layout-A baseline, 162us -> 140us TimelineSim):
  - attn@v runs in [t, d] layout: per (tb, kv-group, head) out = et^T @ vaug
    with M=128 (full PE columns), N=65, head-major so each head's
    start_tensor_calc never wipes a neighbor's partial sums (PSUM zero
    marks are 2KB-region granular).  The softmax denominator lands as a
    per-partition scalar so the division is a cheap DVE tensor op; y is
    PE-transposed back to [d, t] for the output projection.
  - rsqrt for QK-RMSNorm uses the fast-inverse-sqrt bit trick + 2 Newton
    steps on DVE; the only ACT table ever loaded is the Exp set (no Ln,
    which would thrash table reloads at 1.3us each).
  - engine balance: exp + prologue evacuations on ACT; rope/newton/
    y-scale/steady-state evacuations on DVE; ve-gating, leading-edge band
    mask and vaug memsets on GpSimd (Pool; no PSUM access there).
  - the kernel is software-pipelined over 16 (tb, g4) steps: scores(s)
    woven with fillers (kv/q projections, outproj halves), av trailing
    3 steps so PE keeps running while ACT chews the exp stream; input
    DMAs are ordered by first use (single DMA issue queue on SP; issuing
    on the ACT hwdge queue blocks the exp stream and is a net loss).
"""

import sys

for _p in ("/opt/trn_rl_repo",):
    if _p not in sys.path:
        sys.path.insert(0, _p)

import numpy as np
import ml_dtypes

import concourse.bass as bass
import concourse.tile as tile
from concourse import bacc, mybir
from concourse.bass_utils import run_bass_kernel_spmd

BF = ml_dtypes.bfloat16
F8 = mybir.dt.np(mybir.dt.float8e4)
bf16 = mybir.dt.bfloat16
f32 = mybir.dt.float32
i32 = mybir.dt.int32
Act = mybir.ActivationFunctionType
Alu = mybir.AluOpType
X = mybir.AxisListType.X

P = 128
T, C = 4096, 1024
H, HKV, D = 16, 4, 64
GQ = H // HKV            # 4 q heads per kv head
HD = H * D               # 1024
WIN = 1024
NCORE = 8
TLOC = T // NCORE        # 512
S = TLOC + WIN           # 1536 kv rows incl. halo/padding
NSC = S // P             # 12
NTB = TLOC // P          # 4
NCC = C // P             # 8
NB = WIN // P + 1        # 9 s-chunks per 128-row t-block
EPS = float(np.finfo(np.float32).eps)
MAGIC = 0x5EF759E0       # 0x5f3759df - 0x400000 + 1 (rsqrt(2h) from bits(h))


DEBUG_OUT = False


def _build_program():
    nc = bacc.Bacc("TRN2", target_bir_lowering=False, debug=False)

    # chunk-stacked layouts: [p, c*w + j] = orig[c*128 + p, j]
    xT_d = nc.dram_tensor("XT3", [P, 3 * 4096], bf16, kind="ExternalInput")
    wq_d = nc.dram_tensor("WQR", [P, NCC * HD], bf16, kind="ExternalInput")
    wkv_d = nc.dram_tensor("WKVR", [P, NCC * 512], bf16, kind="ExternalInput")
    wg_d = nc.dram_tensor("Wg", [32, 4], bf16, kind="ExternalInput")
    wo_d = nc.dram_tensor("WOR", [P, NCC * C], bf16, kind="ExternalInput")
    ve_d = nc.dram_tensor("VER", [P, NSC * 256], bf16, kind="ExternalInput")
    c2_d = nc.dram_tensor("C2R", [P, NSC * D], bf16, kind="ExternalInput")
    s2_d = nc.dram_tensor("S2R", [P, NSC * D], bf16, kind="ExternalInput")
    con_d = nc.dram_tensor("CON", [P, 3 * P], bf16, kind="ExternalInput")
    db_d = nc.dram_tensor("DBC", [P, NTB], f32, kind="ExternalInput")
    out_d = nc.dram_tensor("out", [TLOC, C], f32, kind="ExternalOutput")
    dbg = None
    if DEBUG_OUT:
        dbg = {
            "RINV": nc.dram_tensor("RINV", [P, 112], f32, kind="ExternalOutput"),
            "KHT": nc.dram_tensor("KHT", [P, 2 * NSC * P], bf16, kind="ExternalOutput"),
            "QHT0": nc.dram_tensor("QHT0", [P, 8 * P], bf16, kind="ExternalOutput"),
            "YH": nc.dram_tensor("YH", [P, 8 * TLOC], bf16, kind="ExternalOutput"),
            "ET1": nc.dram_tensor("ET1", [P, 1024], bf16, kind="ExternalOutput"),
            "YB0": nc.dram_tensor("YB0", [P, 256], bf16, kind="ExternalOutput"),
            "DEN0": nc.dram_tensor("DEN0", [P, 4], f32, kind="ExternalOutput"),
            "VA0": nc.dram_tensor("VA0", [P, 260], bf16, kind="ExternalOutput"),
        }

    with tile.TileContext(nc) as tc:
        _kernel_body(tc, xT_d, wq_d, wkv_d, wg_d, wo_d, ve_d,
                     c2_d, s2_d, con_d, db_d, out_d, dbg)

    nc.compile()
    return nc


def _kernel_body(tc, xT_d, wq_d, wkv_d, wg_d, wo_d, ve_d,
                 c2_d, s2_d, con_d, db_d, out_d, dbg=None):
    nc = tc.nc

    with (
        tc.tile_pool(name="wp", bufs=1) as wp,
        tc.tile_pool(name="work", bufs=6) as work,
        tc.tile_pool(name="qwork", bufs=2) as qwork,
        tc.tile_pool(name="small", bufs=12) as small,
        tc.tile_pool(name="ep", bufs=4) as ep,
        tc.tile_pool(name="ov", bufs=2) as ov,
        tc.tile_pool(name="ps_big", bufs=3, space="PSUM") as ps_big,
        tc.tile_pool(name="ps_av", bufs=1, space="PSUM") as ps_av,
        tc.tile_pool(name="ps_tp", bufs=1, space="PSUM") as ps_tp,
    ):
        # ---- persistent tiles: few big chunk-stacked DMAs ----------------
        # DMA order = first-use order on the engines (single queue, ~3ns/KB):
        # kv0-4 -> rope consts -> q0 (xtb20 + wqh0) -> kv5-8 -> q1 -> rest
        xtb = [[wp.tile([P, 1024], bf16, tag=f"xtb{b}{r}", name=f"xtb{b}{r}")
                for r in range(4)] for b in range(3)]
        wkv4 = [wp.tile([P, 1024], bf16, tag=f"wkv4{h}", name=f"wkv4{h}")
                for h in range(4)]
        wqh = [wp.tile([P, NCC * 512], bf16, tag=f"wqh{h}", name=f"wqh{h}")
               for h in range(2)]
        vert = [wp.tile([P, 4 * 256], bf16, tag=f"vert{v}", name=f"vert{v}")
                for v in range(3)]
        c2re = wp.tile([P, NSC * D], bf16, tag="c2re")
        s2re = wp.tile([P, NSC * D], bf16, tag="s2re")
        con = wp.tile([P, 3 * P], bf16, tag="con")
        dbc = wp.tile([P, NTB], f32, tag="dbc")
        wore = wp.tile([P, NCC * C], bf16, tag="wore")
        wg = wp.tile([32, 4], bf16, tag="wg")
        idt = con[:, 0:P]
        m0t = con[:, P:2 * P]
        m8t = con[:, 2 * P:3 * P]

        def _x(b, r):
            nc.sync.dma_start(xtb[b][r][:],
                              xT_d[:, b * 4096 + r * 1024:b * 4096 + (r + 1) * 1024])

        _x(0, 0)
        nc.sync.dma_start(wkv4[0][:], wkv_d[:, 0:1024])
        nc.sync.dma_start(wg[:], wg_d[:, :])
        _x(0, 1)
        nc.sync.dma_start(c2re[:], c2_d[:, :])
        nc.sync.dma_start(s2re[:], s2_d[:, :])
        nc.sync.dma_start(wkv4[1][:], wkv_d[:, 1024:2048])
        _x(0, 2)
        nc.sync.dma_start(wkv4[2][:], wkv_d[:, 2048:3072])
        _x(0, 3)
        nc.sync.dma_start(wkv4[3][:], wkv_d[:, 3072:4096])
        nc.sync.dma_start(vert[0][:], ve_d[:, 0:1024])
        _x(2, 0)                                   # q rows for tb0/1
        nc.sync.dma_start(wqh[0][:], wq_d[:, 0:4096])
        _x(1, 0)
        _x(1, 1)
        nc.sync.dma_start(wqh[1][:], wq_d[:, 4096:8192])
        nc.sync.dma_start(vert[1][:], ve_d[:, 1024:2048])
        nc.sync.dma_start(con[:], con_d[:, :])
        nc.sync.dma_start(dbc[:], db_d[:, :])
        _x(1, 2)
        _x(1, 3)
        _x(2, 1)
        nc.sync.dma_start(vert[2][:], ve_d[:, 2048:3072])
        _x(2, 2)
        _x(2, 3)
        nc.sync.dma_start(wore[:], wo_d[:, :])

        def xts(c, sc, p0=0, pn=P):
            """lhsT slice of x^T: chunk c, s-chunk sc (r-major layout)."""
            b, r = sc // 4, sc % 4
            return xtb[b][r][p0:pn, c * P:(c + 1) * P]

        # transposed storages
        khT = wp.tile([P, 2 * NSC * P], bf16, tag="khT")     # blk-major
        qhT = [wp.tile([P, 8 * P], bf16, tag=f"qhT{tb}", name=f"qhT{tb}")
               for tb in range(NTB)]
        yh = wp.tile([P, 8 * TLOC], bf16, tag="yh")          # cj-major
        vaug = [wp.tile([P, GQ * 65], bf16, tag=f"vaug{sc}", name=f"vaug{sc}")
                for sc in range(NSC)]

        # rsqrt workspace: cols 0-47 k (4 per sc), 48-111 q (8 per i8)
        ssq = wp.tile([P, 112], f32, tag="ssq")
        hh = wp.tile([P, 112], f32, tag="hh")
        rinv = wp.tile([P, 112], f32, tag="rinv")
        nt0 = wp.tile([P, 112], f32, tag="nt0")
        nt1 = wp.tile([P, 112], f32, tag="nt1")
        nc.vector.memset(ssq[:], 1.0)   # not-yet-written cols stay finite
        nc.vector.memset(hh[:], 1.0)

        rck_tiles = [None] * NSC
        rcq_tiles = [None] * 8

        # ---- helper: rope + ssq ------------------------------------------
        def rope_ssq(src_bf, n_h, sc_rows, rc, ssq_dst, tag):
            """src_bf: [P, n_h*D] bf16 SBUF (pre-rope q or k); rc: bf16 rope
            output; ssq_dst: [P, n_h] f32 slice for sum-of-squares.
            The two sin-halves run on Pool (idle engine), rest on DVE."""
            v3 = src_bf.rearrange("p (h d) -> p h d", d=D)
            c2b = c2re[:, sc_rows * D:(sc_rows + 1) * D].unsqueeze(1).to_broadcast((P, n_h, D))
            nc.vector.tensor_mul(rc[:].rearrange("p (h d) -> p h d", d=D), v3, c2b)
            v4 = src_bf.rearrange("p (h two q) -> p h two q", two=2, q=32)
            rs = work.tile([P, n_h * D], bf16, tag=f"rs{tag}")
            r4 = rs[:].rearrange("p (h two q) -> p h two q", two=2, q=32)
            s2t = s2re[:, sc_rows * D:(sc_rows + 1) * D]
            nc.vector.tensor_mul(
                r4[:, :, 0, :], v4[:, :, 1, :],
                s2t[:, 0:32].unsqueeze(1).to_broadcast((P, n_h, 32)))
            nc.vector.tensor_mul(
                r4[:, :, 1, :], v4[:, :, 0, :],
                s2t[:, 32:64].unsqueeze(1).to_broadcast((P, n_h, 32)))
            nc.vector.tensor_add(rc[:], rc[:], rs[:])
            sq = work.tile([P, n_h * D], bf16, tag=f"sq{tag}")
            nc.vector.tensor_mul(sq[:], rc[:], rc[:])
            nc.vector.reduce_sum(ssq_dst, sq[:].rearrange("p (h d) -> p h d", d=D),
                                 axis=X)

        def kv_chunk(sc, evac="act"):
            # cols 0-511: k|v projection; cols 512-515: ve-gate logits
            kv = ps_big.tile([P, 516], f32, tag="big", name=f"kvp{sc}")
            for c in range(NCC):
                nc.tensor.matmul(
                    kv[:, 0:512], xts(c, sc),
                    wkv4[c // 2][:, (c % 2) * 512:(c % 2 + 1) * 512],
                    start=(c == 0), stop=(c == NCC - 1))
            nc.tensor.matmul(kv[:, 512:516], xts(0, sc, 0, 32), wg[:],
                             start=True, stop=True)
            # sigmoid(x) = 1/(1+exp(-x)) without the Sigmoid table set
            sig = small.tile([P, 4], f32, tag="sig", name=f"sig{sc}")
            nc.scalar.activation(sig[:], kv[:, 512:516], Act.Exp, scale=-1.0)
            nc.vector.tensor_scalar_add(sig[:], sig[:], 1.0)
            nc.vector.reciprocal(sig[:], sig[:])
            # single evacuation; everything downstream reads bf16 SBUF
            kvb = work.tile([P, 512], bf16, tag="kvb", name=f"kvb{sc}")
            if evac == "act":
                nc.scalar.copy(kvb[:], kv[:, 0:512])
            else:
                nc.vector.tensor_copy(kvb[:], kv[:, 0:512])
            # ve-gate on Pool (2 ops) to keep prologue DVE below PE pace
            va = vaug[sc]
            nc.gpsimd.memset(va[:], 1.0)   # ones column (rest overwritten)
            gv = work.tile([P, 256], bf16, tag="gv", name=f"gv{sc}")
            nc.gpsimd.tensor_mul(
                gv[:].rearrange("p (h d) -> p h d", d=D),
                vert[sc // 4][:, (sc % 4) * 256:(sc % 4 + 1) * 256]
                .rearrange("p (h d) -> p h d", d=D),
                sig[:].unsqueeze(2).to_broadcast((P, 4, D)))
            nc.gpsimd.tensor_add(
                va[:].rearrange("p (h e) -> p h e", e=65)[:, :, 0:64],
                gv[:].rearrange("p (h d) -> p h d", d=D),
                kvb[:, 256:512].rearrange("p (h d) -> p h d", d=D))
            rck = wp.tile([P, 256], bf16, tag=f"rck{sc}", name=f"rck{sc}")
            rope_ssq(kvb[:, 0:256], HKV, sc, rck,
                     ssq[:, sc * HKV:(sc + 1) * HKV], "k")
            rck_tiles[sc] = rck

        def q_chunk(i8, evac="act"):
            tb, half = i8 // 2, i8 % 2
            qp = ps_big.tile([P, 512], f32, tag="big", name=f"qp{i8}")
            for c in range(NCC):
                nc.tensor.matmul(
                    qp[:], xts(c, NB - 1 + tb),
                    wqh[half][:, c * 512:(c + 1) * 512],
                    start=(c == 0), stop=(c == NCC - 1))
            qpb = work.tile([P, 512], bf16, tag="qpb", name=f"qpb{i8}")
            if evac == "act":
                nc.scalar.copy(qpb[:], qp[:])
            else:
                nc.vector.tensor_copy(qpb[:], qp[:])
            rcq = qwork.tile([P, 512], bf16, tag=f"rcq{i8 % 4}", name=f"rcq{i8}")
            rope_ssq(qpb[:], 8, NB - 1 + tb, rcq,
                     ssq[:, 48 + i8 * 8:48 + (i8 + 1) * 8], "q")
            rcq_tiles[i8] = rcq

        def newton(lo, hi, tag):
            """rinv[:, lo:hi] = rsqrt(2*hh) from hh = prepared half-args."""
            sl = slice(lo, hi)
            hv, t0, t1, rv = hh[:, sl], nt0[:, sl], nt1[:, sl], rinv[:, sl]
            nc.vector.tensor_scalar(
                t0[:].bitcast(i32), hv.bitcast(i32), 1, None,
                op0=Alu.logical_shift_right)
            nc.vector.tensor_scalar(
                rv.bitcast(i32), t0[:].bitcast(i32), MAGIC - 1, -1,
                op0=Alu.subtract, op1=Alu.mult)
            for _ in range(2):
                nc.vector.tensor_mul(t0, rv, rv)          # y^2
                nc.vector.tensor_mul(t1, t0, hv)          # h y^2
                nc.vector.tensor_scalar(
                    t1, t1, 1.5, -1.0, op0=Alu.subtract, op1=Alu.mult)
                nc.vector.tensor_mul(rv, rv, t1)          # y *= 1.5 - h y^2

        def prep_h(lo, hi, scale):
            sl = slice(lo, hi)
            nc.vector.tensor_scalar(
                hh[:, sl], ssq[:, sl], 64.0 * EPS, scale,
                op0=Alu.add, op1=Alu.mult)

        def rinv_batch(ksc, qi8, tag):
            """ksc: (lo, hi) kv chunk range; qi8: (lo, hi) q i8 range."""
            if ksc[1] > ksc[0]:
                prep_h(ksc[0] * 4, ksc[1] * 4, 1.0 / 128.0)   # 8*rsqrt(ssq)
            if qi8[1] > qi8[0]:
                prep_h(48 + qi8[0] * 8, 48 + qi8[1] * 8, 0.5)  # rsqrt(ssq)
            lo = ksc[0] * 4 if ksc[1] > ksc[0] else 48 + qi8[0] * 8
            hi = 48 + qi8[1] * 8 if qi8[1] > qi8[0] else ksc[1] * 4
            newton(lo, hi, tag)

        def k_fin(sc):
            """scale rck by krinv (one broadcast mul, blk-interleaving dst:
            kv head gi -> col (gi%2)*128 + (gi//2)*64), transpose, evac."""
            khsc = work.tile([P, 256], bf16, tag="khsc", name=f"khsc{sc}")
            nc.vector.tensor_mul(
                khsc[:].rearrange("p (b hh d) -> p hh b d", b=2, hh=2),
                rck_tiles[sc][:].rearrange("p (hh b d) -> p hh b d", b=2, hh=2),
                rinv[:, sc * 4:(sc + 1) * 4]
                .rearrange("p (hh b) -> p hh b", b=2)
                .unsqueeze(3).to_broadcast((P, 2, 2, D)))
            tp = ps_tp.tile([P, 256], bf16, tag="tp", name=f"ktp{sc}")
            nc.tensor.transpose(tp[:, 0:P], khsc[:, 0:P], idt)
            nc.tensor.transpose(tp[:, P:2 * P], khsc[:, P:2 * P], idt)
            dst = khT[:].rearrange("p (b s) -> p b s", b=2)[:, :, sc * P:(sc + 1) * P]
            nc.vector.tensor_copy(dst, tp[:].rearrange("p (b s) -> p b s", b=2))

        def q_fin(tb):
            """scale both q halves by qrinv into slot-interleaved qh, then
            transpose 8 slots (pairs share a psum tile)."""
            qh = qwork.tile([P, HD], bf16, tag="qh", name=f"qh{tb}")
            for half in range(2):
                i8 = tb * 2 + half
                dst = qh[:].rearrange("p (u h d) -> p u h d", h=2, d=D)[:, :, half, :]
                nc.vector.tensor_mul(
                    dst,
                    rcq_tiles[i8][:].rearrange("p (u d) -> p u d", d=D),
                    rinv[:, 48 + i8 * 8:48 + (i8 + 1) * 8]
                    .unsqueeze(2).to_broadcast((P, 8, D)))
            for up in range(4):          # slot pairs (2u, 2u+1)
                tp = ps_tp.tile([P, 256], bf16, tag="tp", name=f"qtp{tb}{up}")
                nc.tensor.transpose(tp[:, 0:P], qh[:, up * 256:up * 256 + P], idt)
                nc.tensor.transpose(tp[:, P:2 * P], qh[:, up * 256 + P:up * 256 + 2 * P], idt)
                nc.vector.tensor_copy(qhT[tb][:, up * 256:(up + 1) * 256], tp[:])

        # ---- attention step pieces ---------------------------------------
        GROUPS = ((0, 1), (1, 2), (3, 2), (5, 2), (7, 2))

        def khs(po, blk, sc):
            return khT[po:po + 64, blk * NSC * P + sc * P:blk * NSC * P + (sc + 1) * P]

        def score_group(tb, g4, gidx, ets):
            """emit one score group's matmuls + exp (ACT) + mask (Pool)."""
            po = (g4 // 2) * 64
            blk = g4 % 2
            u0 = (4 * g4) % 8
            qslc = qhT[tb][po:po + 64, :].rearrange("d (u t) -> d u t", t=P)
            i0, w = GROUPS[gidx]
            sc2 = ps_big.tile([P, 512 * w], f32, tag="big",
                              name=f"sc{tb}{g4}{i0}")
            for k in range(w):
                sc = tb + i0 + k
                nc.tensor.matmul(
                    sc2[:, k * 512:(k + 1) * 512], khs(po, blk, sc),
                    qslc[:, u0:u0 + 4, :],
                    start=True, stop=True)
            et = ep.tile([P, 512 * w], bf16, tag=f"et{gidx}",
                         name=f"et{tb}{g4}{i0}")
            nc.scalar.activation(et[:], sc2[:], Act.Exp)
            if i0 == 0:
                # Pool (idle) takes the leading-edge mask; trailing on DVE
                nc.gpsimd.tensor_mul(
                    et[:, 0:512].rearrange("p (h t) -> p h t", t=P),
                    et[:, 0:512].rearrange("p (h t) -> p h t", t=P),
                    m0t.unsqueeze(1).to_broadcast((P, GQ, P)))
            if i0 + w == NB:
                sl = slice((w - 1) * 512, w * 512)
                nc.vector.tensor_mul(
                    et[:, sl].rearrange("p (h t) -> p h t", t=P),
                    et[:, sl].rearrange("p (h t) -> p h t", t=P),
                    m8t.unsqueeze(1).to_broadcast((P, GQ, P)))
            if dbg is not None and tb == 1 and g4 == 0 and gidx == 1:
                nc.sync.dma_start(dbg["ET1"][:, :], et[:])
            ets.append(et)

        def av_step(tb, g4, ets):
            """layout-B attn@v + denominator + y scale + transpose + evac."""
            # head-major: each head's 9 accumulating matmuls are contiguous so
            # a head's start_tensor_calc (which zero-marks the whole 2KB PSUM
            # region) never wipes another head's partial sums.
            av = ps_av.tile([P, 4 * 65], f32, tag="av", name=f"av{tb}{g4}")
            for hj in range(4):
                for gidx, (i0, w) in enumerate(GROUPS):
                    et = ets[gidx]
                    for k in range(w):
                        i = i0 + k
                        e3 = et[:, k * 512:(k + 1) * 512].rearrange(
                            "p (h t) -> p h t", t=P)
                        nc.tensor.matmul(
                            av[:, hj * 65:(hj + 1) * 65],
                            e3[:, hj, :],
                            vaug[tb + i][:, g4 * 65:(g4 + 1) * 65],
                            start=(i == 0), stop=(i == NB - 1))
            # denominator: av col 64 of each head + padding correction
            av3 = av[:].rearrange("p (h e) -> p h e", e=65)
            den = small.tile([P, 4], f32, tag="den", name=f"den{tb}{g4}")
            nc.vector.tensor_add(
                den[:], av3[:, :, 64],
                dbc[:, tb:tb + 1].to_broadcast((P, 4)))
            nc.vector.reciprocal(den[:], den[:])
            # y = av * rden  (two [P, 2, 64] strided ops), bf16 out
            yb = work.tile([P, 256], bf16, tag="yb", name=f"yb{tb}{g4}")
            for pr in range(2):
                nc.vector.tensor_mul(
                    yb[:].rearrange("p (h d) -> p h d", d=D)[:, pr * 2:pr * 2 + 2, :],
                    av3[:, pr * 2:pr * 2 + 2, 0:64],
                    den[:, pr * 2:pr * 2 + 2].unsqueeze(2).to_broadcast((P, 2, D)))
            if dbg is not None and tb == 1 and g4 == 0:
                nc.sync.dma_start(dbg["YB0"][:, :], yb[:])
                nc.sync.dma_start(dbg["DEN0"][:, :], den[:])
            tp = ps_tp.tile([P, 256], bf16, tag="tp", name=f"ytp{tb}{g4}")
            nc.tensor.transpose(tp[:, 0:P], yb[:, 0:P], idt)
            nc.tensor.transpose(tp[:, P:2 * P], yb[:, P:2 * P], idt)
            # yh layout: cj-major, cj = 2*g4 + pair; 512 t-cols per cj
            dst = yh[:].rearrange("p (cj t) -> p cj t", t=TLOC)[
                :, 2 * g4:2 * g4 + 2, tb * P:(tb + 1) * P]
            nc.vector.tensor_copy(dst, tp[:].rearrange("p (c t) -> p c t", t=P))

        op_tiles = {}

        def outproj_part(tb, half, c0, c1):
            if c0 == 0:
                op_tiles[(tb, half)] = ps_big.tile(
                    [P, 512], f32, tag="big", name=f"op{tb}{half}")
            op = op_tiles[(tb, half)]
            for cj in range(c0, c1):
                nc.tensor.matmul(op[:], yh[:, cj * TLOC + tb * P:
                                             cj * TLOC + (tb + 1) * P],
                                 wore[:, cj * C + half * 512:
                                      cj * C + (half + 1) * 512],
                                 start=(cj == 0), stop=(cj == NCC - 1))
            if c1 == NCC:
                oe = ov.tile([P, 512], f32, tag="oe", name=f"oe{tb}{half}")
                nc.vector.tensor_copy(oe[:], op[:])
                nc.sync.dma_start(
                    out_d[tb * P:(tb + 1) * P, half * 512:(half + 1) * 512],
                    oe[:])

        def outproj_half(tb, half):
            outproj_part(tb, half, 0, NCC)

        # ================= schedule =================
        # prologue: kv 0-8, q 0-1, split newton batches so k_fins/q_fin
        # overlap the later kv chunks instead of serializing at the end
        ETS = {s: [] for s in range(16)}

        def G(s, gidx):
            score_group(s // 4, s % 4, gidx, ETS[s])

        def AV(s):
            av_step(s // 4, s % 4, ETS[s])

        for sc in range(5):
            kv_chunk(sc, evac="act")
        q_chunk(0, evac="act")
        q_chunk(1, evac="act")
        rinv_batch((0, 5), (0, 2), "A1")     # k 0-4 + q i8 0,1
        kv_chunk(5, evac="act")
        k_fin(0)
        k_fin(1)
        kv_chunk(6, evac="act")
        k_fin(2)
        k_fin(3)
        q_fin(0)
        kv_chunk(7, evac="act")
        k_fin(4)
        rinv_batch((5, 7), (0, 0), "A2a")    # k 5,6 (ropes landed)
        kv_chunk(8, evac="act")
        k_fin(5)
        k_fin(6)
        rinv_batch((7, 9), (0, 0), "A2b")    # k 7,8

        FILLERS = {
            0: [lambda: kv_chunk(9, evac="dve"),
                lambda: q_chunk(2, evac="dve")],
            1: [lambda: q_chunk(3, evac="dve"),
                lambda: rinv_batch((9, 10), (2, 4), "B1")],
            2: [lambda: kv_chunk(10, evac="dve"),
                lambda: k_fin(9)],
            3: [lambda: q_fin(1),
                lambda: kv_chunk(11, evac="dve")],
            4: [lambda: q_chunk(4, evac="dve")],
            5: [lambda: q_chunk(5, evac="dve"),
                lambda: rinv_batch((10, 12), (4, 6), "B2")],
            6: [lambda: k_fin(10), lambda: k_fin(11),
                lambda: q_fin(2)],
            7: [lambda: outproj_half(0, 0), lambda: outproj_half(0, 1)],
            8: [lambda: q_chunk(6, evac="dve")],
            9: [lambda: q_chunk(7, evac="dve"),
                lambda: rinv_batch((12, 12), (6, 8), "B3")],
            10: [lambda: q_fin(3)],
            11: [lambda: outproj_half(1, 0)],
            12: [lambda: outproj_half(1, 1)],
            13: [],
            14: [],
            15: [lambda: outproj_half(2, 0)],
        }
        pending = []
        for s in range(16):
            tb, g4 = s // 4, s % 4
            fillers = list(FILLERS[s])
            G(s, 0)
            G(s, 1)
            if s == 0:
                G(0, 2)
                fillers.pop(0)()             # kv9 covers newton A2b latency
                k_fin(7)
                k_fin(8)
            if fillers:
                fillers.pop(0)()
            G(s, 2) if s != 0 else None
            G(s, 3)
            if fillers:
                fillers.pop(0)()
            G(s, 4)
            for f in fillers:
                f()
            pending.append(s)
            if len(pending) > 3:
                AV(pending.pop(0))
        # tail: pipeline op(2)/op(3) around the last three av steps
        AV(pending.pop(0))                   # av(3,1)
        outproj_half(2, 1)
        AV(pending.pop(0))                   # av(3,2)
        outproj_part(3, 0, 0, 6)
        outproj_part(3, 1, 0, 6)
        AV(pending.pop(0))                   # av(3,3)
        outproj_part(3, 0, 6, NCC)
        outproj_part(3, 1, 6, NCC)
        if dbg is not None:
            nc.sync.dma_start(dbg["RINV"][:, :], rinv[:])
            nc.sync.dma_start(dbg["KHT"][:, :], khT[:])
            nc.sync.dma_start(dbg["QHT0"][:, :], qhT[1][:])
            nc.sync.dma_start(dbg["YH"][:, :], yh[:])
            nc.sync.dma_start(dbg["VA0"][:, :], vaug[0][:])


# ---------------------------------------------------------------------------
# host side
# ---------------------------------------------------------------------------

_CACHED = {}


def _program():
    if "nc" not in _CACHED:
        _CACHED["nc"] = _build_program()
    return _CACHED["nc"]


def _prep_core_inputs(core, x, ve, cosp, sinp, shared):
    lo = TLOC * core - WIN
    hi = TLOC * (core + 1)
    pad = max(0, -lo)

    def slc(a):
        s_ = a[max(0, lo):hi]
        if pad:
            s_ = np.concatenate([np.zeros((pad,) + s_.shape[1:], s_.dtype), s_], 0)
        return s_

    xs = slc(x)                                     # [S, C] f32
    A = xs.T                                        # [C, S]
    XT3 = np.ascontiguousarray(
        A.reshape(NCC, P, 3, 4, P).transpose(1, 2, 3, 0, 4).reshape(P, 3 * 4096)
    ).astype(BF)
    VER = _stack(2.0 * slc(ve)).astype(BF)
    cs = slc(cosp)                                  # [S, 32]
    sn = slc(sinp)
    C2 = _stack(np.concatenate([cs, cs], 1).astype(np.float32)).astype(BF)
    S2 = _stack(np.concatenate([sn, -sn], 1).astype(np.float32)).astype(BF)

    tl = np.arange(TLOC)
    npad = np.maximum(0, np.minimum(WIN + 1, pad - tl)).astype(np.float32)
    DBC = np.ascontiguousarray((-npad).reshape(NTB, P).T)   # [P, NTB]

    m = dict(shared)
    m.update({"XT3": XT3, "VER": VER, "C2R": C2, "S2R": S2, "DBC": DBC})
    return m


def kernel(x, ve, cos, sin, Wq, Wk, Wv, Wo, Wg, window_size):
    out, _ = _run(x, ve, cos, sin, Wq, Wk, Wv, Wo, Wg, window_size)
    return out


def _stack(A):
    """[n*128, w] -> [128, n*w] with [p, c*w+j] = A[c*128+p, j]."""
    n = A.shape[0] // P
    return np.ascontiguousarray(
        A.reshape(n, P, A.shape[1]).transpose(1, 0, 2).reshape(P, -1))


def _shared_inputs(Wq, Wk, Wv, Wo, Wg):
    ar = np.arange(P)
    # Wo rows in natural head order: chunk cj = heads (2cj, 2cj+1)
    wo_re = np.asarray(Wo, np.float32)
    wkv = np.concatenate([np.asarray(Wk, np.float32),
                          np.asarray(Wv, np.float32)], 1)
    con = np.concatenate(
        [np.eye(P, dtype=np.float32),
         (ar[:, None] >= ar[None, :]).astype(np.float32),
         (ar[:, None] <= ar[None, :]).astype(np.float32)], 1)
    # WQR half-major: [p, half*4096 + c*512 + j]
    wq_s = _stack(np.asarray(Wq, np.float32))        # [P, NCC*1024]
    wq2 = wq_s.reshape(P, NCC, 2, 512).transpose(0, 2, 1, 3).reshape(P, -1)
    return {
        "WQR": np.ascontiguousarray(wq2).astype(BF),
        "WKVR": _stack(wkv).astype(BF),
        "Wg": np.asarray(Wg, np.float32).astype(BF),
        "WOR": _stack(wo_re).astype(BF),
        "CON": np.ascontiguousarray(con).astype(BF),
    }


def _run(x, ve, cos, sin, Wq, Wk, Wv, Wo, Wg, window_size, trace=False):
    assert int(window_size) == WIN
    x = np.asarray(x, np.float32)[0]                # [T, C]
    ve_ = np.asarray(ve, np.float32)[0]             # [T, 256]
    cosp = np.asarray(cos, np.float32)[0, :, 0, :]  # [T, 32]
    sinp = np.asarray(sin, np.float32)[0, :, 0, :]

    shared = _shared_inputs(Wq, Wk, Wv, Wo, Wg)

    in_maps = [_prep_core_inputs(i, x, ve_, cosp, sinp, shared)
               for i in range(NCORE)]
    nc = _program()
    res = run_bass_kernel_spmd(nc, in_maps, core_ids=list(range(NCORE)),
                               trace=trace)
    out = np.concatenate([res.results[i]["out"] for i in range(NCORE)], 0)
    return out.reshape(1, T, C).astype(np.float32), res
